# revision 67
# baseline (speedup 1.0000x reference)
"""Trainium2 Bass kernel for GNN message-passing attention block.

Strategy: shard queries (n axis) 8 ways. On device everything is dense:
  GroupNorm -> Q/K/V projections (PE) -> dense scores S_h = K_h^T Q_h (PE)
  -> exp (ACT) -> multiply by host-precomputed multiplicity/validity mask
  M[j,q] (DVE) -> V^T @ [A] matmul with an appended ones-column giving both
  the weighted sum and the softmax normalizer Z (PE) -> divide -> output
  projection + residual.

The sparse gather/scatter of the reference collapses into the dense mask M
because duplicate neighbor indices share the same score: their combined
softmax weight is multiplicity * exp(s) / Z.
"""

import sys

if "/opt/trn_rl_repo" not in sys.path:
    sys.path.insert(0, "/opt/trn_rl_repo")

import numpy as np

import concourse.bacc as bacc
import concourse.mybir as mybir
import concourse.tile as tile
from concourse import bass_utils
from contextlib import ExitStack

F32 = mybir.dt.float32
BF16 = mybir.dt.bfloat16
AF = mybir.ActivationFunctionType
ALU = mybir.AluOpType
AX = mybir.AxisListType

B, C, N, K, H, DH, NG = 2, 512, 1024, 64, 8, 64, 32
NQ = 128            # queries per core
NCHUNK = C // 128   # 4 channel chunks
NJC = N // 128      # 8 key-node chunks
EPS = 1e-6
GSIZE = (C // NG) * N  # elements per group = 16*1024

_CACHE = {}
LAST_EXEC_NS = None
TRACE = False


def _emit(tc, nc, t):
    """Emit the per-core program, stage-pipelined across the two batches so
    every engine's in-order queue interleaves batch 0 and batch 1 work."""
    ctx = t["ctx"]
    P = 128

    wpool = ctx.enter_context(tc.tile_pool(name="weights", bufs=1))
    cpool = ctx.enter_context(tc.tile_pool(name="consts", bufs=1))
    xpool = ctx.enter_context(tc.tile_pool(name="x", bufs=2))
    hpool = ctx.enter_context(tc.tile_pool(name="h", bufs=2))
    kvpool = ctx.enter_context(tc.tile_pool(name="kv", bufs=2))
    apool = ctx.enter_context(tc.tile_pool(name="attn", bufs=2))
    spool = ctx.enter_context(tc.tile_pool(name="scratch", bufs=2))
    smallp = ctx.enter_context(tc.tile_pool(name="small", bufs=2))
    opool = ctx.enter_context(tc.tile_pool(name="out", bufs=2))
    # PSUM budget (8 banks of 2KB): pp "mm" 1 bank x2, ps_pool 1 bank x2,
    # pp1 po 2 banks x2 = 8 banks exactly.
    pp = ctx.enter_context(tc.tile_pool(name="psum", bufs=2, space="PSUM"))
    ps_pool = ctx.enter_context(tc.tile_pool(name="psum_s", bufs=2, space="PSUM"))
    pp1 = ctx.enter_context(tc.tile_pool(name="psum1", bufs=2, space="PSUM"))

    # ---- constants first (tiny); weight/mask loads deferred until after the
    # x loads so GroupNorm stats can start ASAP ----
    consts = cpool.tile([P, 21], F32)  # bq|bk|gamma|beta|bo_eff  (4 cols each)
    nc.sync.dma_start(consts[:], t["consts"][:, :])
    gfwd = cpool.tile([P, 8], F32)
    gbwd = cpool.tile([8, P], F32)
    nc.sync.dma_start(gfwd[:], t["gfwd"][:, :])
    nc.sync.dma_start(gbwd[:], t["gbwd"][:, :])
    wqT = [wpool.tile([P, C], BF16, tag=f"wqT{i}", name=f"wqT{i}") for i in range(NCHUNK)]
    wkT = [wpool.tile([P, C], BF16, tag=f"wkT{i}", name=f"wkT{i}") for i in range(NCHUNK)]
    wvT = [wpool.tile([P, C], BF16, tag=f"wvT{i}", name=f"wvT{i}") for i in range(NCHUNK)]
    woT = [wpool.tile([P, C], BF16, tag=f"woT{i}", name=f"woT{i}") for i in range(NCHUNK)]
    msb = [cpool.tile([P, NQ], BF16, tag=f"m{jc}", name=f"m{jc}") for jc in range(NJC)]
    ones64 = cpool.tile([1, DH], F32)
    nc.vector.memset(ones64[:], 1.0)

    S = [dict() for _ in range(B)]

    # ---- stage 0: x loads (b0 first, then weights, then b1) ----
    for b in range(B):
        d = S[b]
        d["xsb"] = [xpool.tile([P, N], BF16, tag=f"x{m}", name=f"x{m}_{b}") for m in range(NCHUNK)]
        d["xq"] = [xpool.tile([P, NQ], F32, tag=f"xq{m}", name=f"xq{m}_{b}") for m in range(NCHUNK)]
        for m in range(NCHUNK):
            sl = slice(m * 128, (m + 1) * 128)
            nc.sync.dma_start(d["xsb"][m][:], t["x"][b, sl, :])
        for m in range(NCHUNK):
            sl = slice(m * 128, (m + 1) * 128)
            nc.sync.dma_start(d["xq"][m][:], t["xq"][b, sl, :])
        if b == 0:
            for i in range(NCHUNK):
                sl = slice(i * 128, (i + 1) * 128)
                nc.scalar.dma_start(wqT[i][:], t["wqT"][sl, :])
                nc.scalar.dma_start(wkT[i][:], t["wkT"][sl, :])
                nc.gpsimd.dma_start(wvT[i][:], t["wvT"][sl, :])
                nc.gpsimd.dma_start(woT[i][:], t["woT"][sl, :])
        else:
            for jc in range(NJC):
                nc.gpsimd.dma_start(msb[jc][:], t["mmask"][jc, :, :])

    # ---- stage 1: GroupNorm statistics ----
    for b in range(B):
        d = S[b]
        xsb, xq = d["xsb"], d["xq"]
        # bn_stats one-pass per-partition (mean, var); group-aggregate via PE
        ssq = smallp.tile([P, 8], F32, tag="ssq", name=f"ssq_{b}")  # cols 0-3 mean, 4-7 E[x^2]
        bnraw = smallp.tile([P, 2, 6], F32, tag="bnraw", name=f"bnraw_{b}")
        bnag = smallp.tile([P, 2], F32, tag="bnag", name=f"bnag_{b}")
        for m in range(NCHUNK):
            for tblk in range(2):
                nc.vector.bn_stats(bnraw[:, tblk, :],
                                   xsb[m][:, tblk * 512:(tblk + 1) * 512])
            nc.vector.bn_aggr(bnag[:], bnraw[:])
            # ssq[:, m] = mean_p ; ssq[:, 4+m] = E[x^2]_p = var_p + mean_p^2
            nc.vector.tensor_copy(ssq[:, m:m + 1], bnag[:, 0:1])
            nc.vector.scalar_tensor_tensor(ssq[:, 4 + m:5 + m], bnag[:, 0:1],
                                           bnag[:, 0:1], bnag[:, 1:2],
                                           ALU.mult, ALU.add)
        gs = pp.tile([8, 8], F32, tag="mm", name=f"gs_{b}")
        nc.tensor.matmul(gs[:], gfwd[:], ssq[:], start=True, stop=True)
        mu = smallp.tile([8, 8], F32, tag="mu", name=f"mu_{b}")
        nc.scalar.activation(mu[:], gs[:], AF.Copy, scale=1.0 / 16.0)
        var = smallp.tile([8, 4], F32, tag="var", name=f"var_{b}")
        nc.vector.tensor_tensor(var[:], mu[:, 0:4], mu[:, 0:4], ALU.mult)
        nc.vector.tensor_tensor(var[:], mu[:, 4:8], var[:], ALU.subtract)
        sd = smallp.tile([8, 4], F32, tag="sd", name=f"sd_{b}")
        nc.vector.tensor_scalar_add(sd[:], var[:], EPS)
        sdq = smallp.tile([8, 4], F32, tag="sdq", name=f"sdq_{b}")
        nc.scalar.activation(sdq[:], sd[:], AF.Sqrt)
        rs = smallp.tile([8, 4], F32, tag="rs", name=f"rs_{b}")
        nc.vector.reciprocal(rs[:], sdq[:])
        bc = pp.tile([P, 8], F32, tag="mm", name=f"bc_{b}")
        nc.tensor.matmul(bc[:, 0:4], gbwd[:], mu[:, 0:4], start=True, stop=True)
        nc.tensor.matmul(bc[:, 4:8], gbwd[:], rs[:], start=True, stop=True)
        ga = smallp.tile([P, 4], F32, tag="ga", name=f"ga_{b}")
        gb = smallp.tile([P, 4], F32, tag="gb", name=f"gb_{b}")
        nc.vector.tensor_tensor(ga[:], consts[:, 8:12], bc[:, 4:8], ALU.mult)
        nc.vector.tensor_tensor(gb[:], bc[:, 0:4], ga[:], ALU.mult)
        nc.vector.tensor_tensor(gb[:], consts[:, 12:16], gb[:], ALU.subtract)
        d["ga"], d["gb"] = ga, gb

    # ---- stage 2: GN apply + Q/K/V projections ----
    for b in range(B):
        d = S[b]
        xsb, xq, ga, gb = d["xsb"], d["xq"], d["ga"], d["gb"]
        hsb = [hpool.tile([P, N], BF16, tag=f"h{m}", name=f"h{m}_{b}") for m in range(NCHUNK)]
        hq = [hpool.tile([P, NQ], BF16, tag=f"hq{m}", name=f"hq{m}_{b}") for m in range(NCHUNK)]
        for m in range(NCHUNK):
            nc.vector.scalar_tensor_tensor(
                hq[m][:], xq[m][:], ga[:, m:m + 1],
                gb[:, m:m + 1].broadcast_to([P, NQ]), ALU.mult, ALU.add)
        for m in range(NCHUNK):
            nc.vector.scalar_tensor_tensor(
                hsb[m][:], xsb[m][:], ga[:, m:m + 1],
                gb[:, m:m + 1].broadcast_to([P, N]), ALU.mult, ALU.add)
        qsb = [kvpool.tile([P, NQ], BF16, tag=f"q{mo}", name=f"q{mo}_{b}") for mo in range(NCHUNK)]
        ksb = [kvpool.tile([P, N], BF16, tag=f"k{mo}", name=f"k{mo}_{b}") for mo in range(NCHUNK)]
        for mo in range(NCHUNK):
            osl = slice(mo * 128, (mo + 1) * 128)
            pq = pp.tile([P, NQ], F32, tag="mm", name=f"pq_{b}")
            for ki in range(NCHUNK):
                nc.tensor.matmul(pq[:], wqT[ki][:, osl], hq[ki][:],
                                 start=(ki == 0), stop=(ki == NCHUNK - 1))
            nc.scalar.activation(qsb[mo][:], pq[:], AF.Identity,
                                 bias=consts[:, mo:mo + 1])
            for nt in range(2):
                nsl = slice(nt * 512, (nt + 1) * 512)
                pk = pp.tile([P, 512], F32, tag="mm", name=f"pk_{b}")
                for ki in range(NCHUNK):
                    nc.tensor.matmul(pk[:], wkT[ki][:, osl], hsb[ki][:, nsl],
                                     start=(ki == 0), stop=(ki == NCHUNK - 1))
                nc.scalar.activation(ksb[mo][:, nsl], pk[:], AF.Identity,
                                     bias=consts[:, 4 + mo:5 + mo])
        vT = [kvpool.tile([P, H, DH + 1], BF16, tag=f"vT{jc}", name=f"vT{jc}_{b}") for jc in range(NJC)]
        for jc in range(NJC):
            jsl = slice(jc * 128, (jc + 1) * 128)
            pv = pp.tile([P, C], F32, tag="mm", name=f"pv_{b}")
            for ki in range(NCHUNK):
                nc.tensor.matmul(pv[:], hsb[ki][:, jsl], wvT[ki][:],
                                 start=(ki == 0), stop=(ki == NCHUNK - 1))
            nc.vector.tensor_copy(vT[jc][:, :, 0:DH],
                                  pv[:].rearrange("p (h d) -> p h d", h=H))
            nc.gpsimd.memset(vT[jc][:, :, DH:DH + 1], 1.0)
        # odd-head halves of Q/K to partition-0 tiles via SBUF->SBUF DMA
        # (matmul operands at partition offset 64 crash hardware)
        qod = [kvpool.tile([64, NQ], BF16, tag=f"qo{mo}", name=f"qo{mo}_{b}") for mo in range(NCHUNK)]
        kod = [kvpool.tile([64, N], BF16, tag=f"ko{mo}", name=f"ko{mo}_{b}") for mo in range(NCHUNK)]
        for mo in range(NCHUNK):
            nc.gpsimd.dma_start(qod[mo][:], qsb[mo][64:128, :])
            nc.gpsimd.dma_start(kod[mo][:], ksb[mo][64:128, :])
        d.update(qsb=qsb, ksb=ksb, vT=vT, qod=qod, kod=kod)

    # ---- stage 3: attention (scores -> exp -> mask -> AV+Z matmul) ----
    for b in range(B):
        d = S[b]
        qsb, ksb, vT, qod, kod = d["qsb"], d["ksb"], d["vT"], d["qod"], d["kod"]
        po = pp1.tile([DH + 1, H * NQ], F32, tag="po", name=f"po_{b}")
        asb = [apool.tile([P, H * NQ], BF16, tag=f"a{jc}", name=f"a{jc}_{b}") for jc in range(NJC)]
        for jc in range(NJC):
            jsl = slice(jc * 128, (jc + 1) * 128)
            for half in range(2):
                ps = ps_pool.tile([P, 4 * NQ], F32, tag="ps", name=f"ps_{b}")
                for hh in range(4):
                    h = half * 4 + hh
                    mo = h // 2
                    if h % 2 == 0:
                        nc.tensor.matmul(ps[:, hh * NQ:(hh + 1) * NQ],
                                         ksb[mo][0:64, jsl],
                                         qsb[mo][0:64, :],
                                         start=True, stop=True)
                    else:
                        nc.tensor.matmul(ps[:, hh * NQ:(hh + 1) * NQ],
                                         kod[mo][:, jsl],
                                         qod[mo][:],
                                         start=True, stop=True)
                nc.scalar.activation(
                    asb[jc][:, half * 4 * NQ:(half + 1) * 4 * NQ],
                    ps[:], AF.Exp)
            nc.vector.tensor_tensor(
                asb[jc][:].rearrange("p (h q) -> p h q", h=H),
                asb[jc][:].rearrange("p (h q) -> p h q", h=H),
                msb[jc][:].rearrange("p (o q) -> p o q", o=1).broadcast_to([P, H, NQ]),
                ALU.mult)
        zinv = smallp.tile([1, H * NQ], F32, tag="zinv", name=f"zinv_{b}")
        for half in range(2):
            for hh in range(4):
                h = half * 4 + hh
                for jc in range(NJC):
                    nc.tensor.matmul(po[:, h * NQ:(h + 1) * NQ],
                                     vT[jc][:, h, :],
                                     asb[jc][:, h * NQ:(h + 1) * NQ],
                                     start=(jc == 0), stop=(jc == NJC - 1))
            hsl = slice(half * 4 * NQ, (half + 1) * 4 * NQ)
            nc.vector.reciprocal(zinv[:, hsl], po[DH:DH + 1, hsl])
        d["po"] = po
        d["zinv"] = zinv

    # ---- stage 4: normalize, output projection, residual, store ----
    for b in range(B):
        d = S[b]
        po, xq, zinv = d["po"], d["xq"], d["zinv"]
        zbc = spool.tile([DH, H * NQ], F32, tag="zbc", name=f"zbc_{b}")
        osb = [opool.tile([P, NQ], BF16, tag=f"o{mo}", name=f"o{mo}_{b}") for mo in range(NCHUNK)]
        for half in range(2):
            nsl = slice(half * 512, (half + 1) * 512)
            pz = pp.tile([DH, 512], F32, tag="mm", name=f"pz_{b}")
            nc.tensor.matmul(pz[:], ones64[:], zinv[:, nsl], start=True, stop=True)
            nc.scalar.activation(zbc[:, nsl], pz[:], AF.Copy)
            for hh in range(4):
                h = half * 4 + hh
                mo, poff = h // 2, (h % 2) * 64
                nc.vector.tensor_tensor(
                    osb[mo][poff:poff + 64, :],
                    po[0:DH, h * NQ:(h + 1) * NQ],
                    zbc[0:DH, h * NQ:(h + 1) * NQ],
                    ALU.mult)
        for mo in range(NCHUNK):
            osl = slice(mo * 128, (mo + 1) * 128)
            py = pp.tile([P, NQ], F32, tag="mm", name=f"py_{b}")
            for ki in range(NCHUNK):
                nc.tensor.matmul(py[:], woT[ki][:, osl], osb[ki][:],
                                 start=(ki == 0), stop=(ki == NCHUNK - 1))
            ysb = opool.tile([P, NQ], F32, tag="y", name=f"y_{b}")
            nc.vector.scalar_tensor_tensor(ysb[:], py[:], consts[:, 16 + mo:17 + mo],
                                           xq[mo][:], ALU.add, ALU.add)
            nc.sync.dma_start(t["y"][b, osl, :], ysb[:])


def _build():
    nc = bacc.Bacc("TRN2", target_bir_lowering=False, debug=False, num_devices=8)
    t = {}
    t["x"] = nc.dram_tensor("x", [B, C, N], BF16, kind="ExternalInput").ap()
    t["xq"] = nc.dram_tensor("xq", [B, C, NQ], F32, kind="ExternalInput").ap()
    t["mmask"] = nc.dram_tensor("mmask", [NJC, 128, NQ], BF16,
                                kind="ExternalInput").ap()
    for w in ("wqT", "wkT", "wvT", "woT"):
        t[w] = nc.dram_tensor(w, [C, C], BF16, kind="ExternalInput").ap()
    t["consts"] = nc.dram_tensor("consts", [128, 21], F32, kind="ExternalInput").ap()
    t["gfwd"] = nc.dram_tensor("gfwd", [128, 8], F32, kind="ExternalInput").ap()
    t["gbwd"] = nc.dram_tensor("gbwd", [8, 128], F32, kind="ExternalInput").ap()
    t["y"] = nc.dram_tensor("y", [B, C, NQ], F32, kind="ExternalOutput").ap()
    with tile.TileContext(nc) as tc, ExitStack() as ctx:
        t["ctx"] = ctx
        _emit(tc, nc, t)
    nc.compile()
    return nc


def _prep_inputs(inputs):
    x = np.ascontiguousarray(np.asarray(inputs["x"], dtype=np.float32))
    idx = np.asarray(inputs["attend_idx"]).astype(np.int64)
    vm = np.asarray(inputs["valid_mask"]).astype(np.float32)
    wq = np.asarray(inputs["wq"], dtype=np.float32)
    wk = np.asarray(inputs["wk"], dtype=np.float32)
    wv = np.asarray(inputs["wv"], dtype=np.float32)
    wo = np.asarray(inputs["wo"], dtype=np.float32)
    bq = np.asarray(inputs["bq"], dtype=np.float32)
    bk = np.asarray(inputs["bk"], dtype=np.float32)
    bv = np.asarray(inputs["bv"], dtype=np.float32)
    bo = np.asarray(inputs["bo"], dtype=np.float32)
    gamma = np.asarray(inputs["gn_gamma"], dtype=np.float32)
    beta = np.asarray(inputs["gn_beta"], dtype=np.float32)

    cols = np.arange(C)
    perm = (cols % DH) * H + cols // DH   # wo_perm[:, h*64+d] = wo[:, d*8+h]
    wo_perm = wo[:, perm]
    bo_eff = bo + wo_perm @ bv

    def colmajor(v):
        return np.ascontiguousarray(v.reshape(NCHUNK, 128).T)

    consts = np.concatenate(
        [colmajor(v) for v in (bq, bk, gamma, beta, bo_eff)]
        + [np.full((128, 1), EPS, np.float32)], axis=1)
    gfwd = np.zeros((128, 8), np.float32)
    gfwd[np.arange(128), np.arange(128) // 16] = 1.0
    gbwd = np.ascontiguousarray(gfwd.T)

    from ml_dtypes import bfloat16
    shared = {
        "x": x.astype(bfloat16),
        "wqT": np.ascontiguousarray(wq.T).astype(bfloat16),
        "wkT": np.ascontiguousarray(wk.T).astype(bfloat16),
        "wvT": np.ascontiguousarray(wv.T).astype(bfloat16),
        "woT": np.ascontiguousarray(wo_perm.T).astype(bfloat16),
        "consts": np.ascontiguousarray(consts),
        "gfwd": gfwd,
        "gbwd": gbwd,
    }
    in_maps = []
    for r in range(8):
        qs = slice(r * NQ, (r + 1) * NQ)
        Mr = np.zeros((N, NQ), np.float32)
        np.add.at(Mr, (idx[qs].ravel(), np.repeat(np.arange(NQ), K)),
                  vm[qs].ravel())
        m = dict(shared)
        m["mmask"] = np.ascontiguousarray(Mr.reshape(NJC, 128, NQ)).astype(bfloat16)
        m["xq"] = np.ascontiguousarray(x[:, :, qs])
        in_maps.append(m)
    return in_maps


def _get_runner(n_cores=8):
    """Build (once) a cached jitted SPMD executor mirroring
    bass2jax.run_bass_via_pjrt, so repeated calls don't re-trace."""
    if "runner" in _CACHE:
        return _CACHE["runner"]
    if "nc" not in _CACHE:
        _CACHE["nc"] = _build()
    nc = _CACHE["nc"]
    import jax
    from jax.sharding import Mesh, PartitionSpec
    from jax.experimental.shard_map import shard_map
    from concourse import bass2jax
    import concourse.mybir as _mybir

    bass2jax.install_neuronx_cc_hook()
    part_name = nc.partition_id_tensor.name if nc.partition_id_tensor else None
    in_names, out_names, out_avals, zero_outs = [], [], [], []
    for alloc in nc.m.functions[0].allocations:
        if not isinstance(alloc, _mybir.MemoryLocationSet):
            continue
        name = alloc.memorylocations[0].name
        if alloc.kind == "ExternalInput":
            if name != part_name:
                in_names.append(name)
        elif alloc.kind == "ExternalOutput":
            shape = tuple(alloc.tensor_shape)
            dtype = _mybir.dt.np(alloc.dtype)
            out_names.append(name)
            out_avals.append(jax.core.ShapedArray(shape, dtype))
            zero_outs.append(np.zeros(shape, dtype))
    n_params = len(in_names)
    n_outs = len(out_avals)
    all_names = in_names + out_names
    if part_name is not None:
        all_names = all_names + [part_name]
    donate = tuple(range(n_params, n_params + n_outs))

    def _body(*args):
        operands = list(args)
        if part_name is not None:
            operands.append(bass2jax.partition_id_tensor())
        outs = bass2jax._bass_exec_p.bind(
            *operands,
            out_avals=tuple(out_avals),
            in_names=tuple(all_names),
            out_names=tuple(out_names),
            lowering_input_output_aliases=(),
            sim_require_finite=True,
            sim_require_nnan=True,
            nc=nc,
        )
        return tuple(outs)

    devices = jax.devices()[:n_cores]
    mesh = Mesh(np.asarray(devices), ("core",))
    fn = jax.jit(
        shard_map(_body, mesh=mesh,
                  in_specs=(PartitionSpec("core"),) * (n_params + n_outs),
                  out_specs=(PartitionSpec("core"),) * n_outs,
                  check_rep=False),
        donate_argnums=donate, keep_unused=True)

    def run(in_maps, device_inputs=None):
        if device_inputs is None:
            device_inputs = put_inputs(in_maps)
        zo = [np.concatenate([np.zeros_like(z)] * n_cores, axis=0)
              for z in zero_outs]
        outs = fn(*device_inputs, *zo)
        outs = [np.asarray(o) for o in outs]
        split = [np.split(o, n_cores, axis=0) for o in outs]
        return [{name: split[i][c] for i, name in enumerate(out_names)}
                for c in range(n_cores)]

    def put_inputs(in_maps):
        cat = [np.concatenate([np.asarray(in_maps[c][nm])
                               for c in range(n_cores)], axis=0)
               for nm in in_names]
        return [jax.device_put(a) for a in cat]

    _CACHE["runner"] = (run, put_inputs, fn, n_params, n_outs)
    return _CACHE["runner"]


def _sim_fallback(nc, in_maps):
    """Correctness fallback if the PJRT/hardware path errors: run each
    core's shard through CoreSim (validated at ~5e-7 rel err)."""
    from concourse.bass_interp import CoreSim
    results = []
    for m in in_maps:
        sim = CoreSim(nc, require_finite=False)
        for k, v in m.items():
            sim.tensor(k)[:] = v
        sim.simulate(check_with_hw=False)
        results.append({"y": np.array(sim.tensor("y"))})
    return results


def kernel(**inputs):
    in_maps = _prep_inputs(inputs)
    try:
        run, put_inputs, _, _, _ = _get_runner()
        results = run(in_maps)
    except Exception as e:
        sys.stderr.write(f"kernel: hardware path failed ({e!r}); "
                         "falling back to CoreSim\n")
        results = _sim_fallback(_CACHE["nc"], in_maps)
    out = np.concatenate([np.asarray(results[r]["y"]) for r in range(8)],
                         axis=2)
    return np.ascontiguousarray(out.astype(np.float32))

